# revision 50
# baseline (speedup 1.0000x reference)
"""Multi-head causal self-attention (B=2, T=4096, C=768, H=12, D=64) on 8 NeuronCores.

Sharding: core c handles batch b = c // 4 and a group of 3 heads (c % 4).
Each core runs a fused flash-attention pipeline per 512-column tq chunk:
QKV projection -> V transpose -> streaming softmax(QK^T)V -> output projection,
producing a partial (pre-bias) out.T [768, 4096]. The host sums the 4 partials
per batch and adds the projection bias.

Softmax is computed without max-subtraction (scores are O(+-6), well inside
fp32 exp range); the denominator comes for free from an appended ones-column
in V. exp() is split across engines: diagonal (masked) pairs use exact exp on
the Activation engine; off-diagonal pairs are split between Activation-exp and
a Schraudolph approximation on the Vector engine (int32(x*A+B) bit-cast is a
~2% accurate fp32 exp; the attnV matmul reads its top 16 bits as bf16 via a
stride-2 access pattern, so no extra conversion pass is needed).
All matmuls run as fp32r (PE reduced-precision fp32) with fp32 accumulation.
"""

from contextlib import ExitStack

import numpy as np

import concourse.bass as bass
import concourse.tile as tile
from concourse import bacc
from concourse import mybir
from concourse._compat import with_exitstack
from concourse.bass_utils import run_bass_kernel_spmd

F32 = mybir.dt.float32
F32R = mybir.dt.float32r
BF16 = mybir.dt.bfloat16
I32 = mybir.dt.int32
EXP = mybir.ActivationFunctionType.Exp
LN = mybir.ActivationFunctionType.Ln
IDENT = mybir.ActivationFunctionType.Identity
MULT = mybir.AluOpType.mult
ADD = mybir.AluOpType.add

B, T, C = 2, 4096, 768
H, D = 12, 64
NCORES = 8
HPC = 3           # heads per core
GPB = NCORES // B  # head-group cores per batch (4)
TQ = 512          # tq chunk width
NJ = T // TQ      # 8
TKB = 128         # tk block
NB = T // TKB     # 32
KC = C // 128     # 6 contraction chunks for the QKV projection
SCALE = 1.0 / np.sqrt(D)

# Schraudolph exp: exp(x) ~= bitcast_f32(int32(x * 2^23/ln2 + (127<<23) - CADJ))
SCH_A = 12102203.0
SCH_CADJ = 366393.0
SCH_B = float((127 << 23)) - SCH_CADJ
RECIP_MAGIC = 0x7EF311C3  # bit-affine 1/x seed; one Newton step -> +-0.26%

import os
SCH_ENABLE = os.environ.get("SCH_ENABLE", "1") == "1"
PIPE_ENABLE = os.environ.get("PIPE_ENABLE", "1") == "1"
RECIP_FAST = os.environ.get("RECIP_FAST", "1") == "1"

# Layout of the per-core QKV weight columns: 5 chunks of 128 (last half-used).
# Each entry is (quantity, local head, partition base within the chunk).
# Chosen so Q and K of the same head land on the same partition half (their
# zero-padded halves line up in the 128-deep score contraction), and V lands
# where its transpose is convenient.
CHUNKS = [
    [("Q", 0, 0), ("Q", 1, 64)],
    [("Q", 2, 0), ("V", 0, 64)],
    [("K", 0, 0), ("K", 1, 64)],
    [("K", 2, 0), ("V", 1, 64)],
    [("V", 2, 0)],
]
NQKV = 4 * 128 + 64  # 576 columns of per-core qkv weights

QK_BASE = {0: 0, 1: 64, 2: 0}   # partition base of Q/K data for score matmuls
V_BASE = {0: 64, 1: 64, 2: 0}   # partition base of V.T data in its stage tile
# Vp block layout [128 keys, 128 cols]: per head, V data occupies VCOL..VCOL+64
# and a ones-column at ONES_COL supplies the softmax denominator. attnV output
# partitions = lhsT free index, so heads 0/1 land their O at rows 64:128 and
# head 2 at rows 0:64 -> heads pack into two 128-row ot slots for the output
# projection (slot0 = h2 rows 0:64 + h0 rows 64:128; slot1 = h1 rows 64:128).
VCOL = {0: 64, 1: 64, 2: 0}
ONES_COL = {0: 32, 1: 32, 2: 64}  # 32-aligned (engine partition-base rule)
LHS_LO = {0: 0, 1: 0, 2: 0}     # attnV lhsT column window start
LHS_W = {0: 128, 1: 128, 2: 65}  # attnV lhsT free width
DEN_ROW = {0: 32, 1: 32, 2: 64}  # pso row holding the denominator
O_ROW = {0: 64, 1: 64, 2: 0}    # pso/ot row base of the 64 output dims
OT_SLOT = {0: 0, 1: 1, 2: 0}    # ot slot per head
NSLOT = 2


def _proj(nc, ps_misc, stp, wp_sb, outT_r, ot, j):
    jsl = slice(j * TQ, (j + 1) * TQ)
    for m in range(KC):
        ps3 = ps_misc.tile([128, TQ], F32, tag="misc", name="ps3")
        for sl in range(NSLOT):
            nc.tensor.matmul(
                ps3[:],
                lhsT=wp_sb[:, sl, m * 128:(m + 1) * 128],
                rhs=ot[:, sl, :],
                start=(sl == 0),
                stop=(sl == NSLOT - 1),
            )
        st = stp.tile([128, TQ], F32, tag="st", name="st")
        if m % 2 == 0:
            nc.vector.tensor_copy(st[:], ps3[:])
        else:
            nc.scalar.copy(st[:], ps3[:])
        nc.gpsimd.dma_start(outT_r[:, m, jsl], st[:])


@with_exitstack
def _mhsa_body(ctx: ExitStack, tc: tile.TileContext, t):
    nc = tc.nc
    xT_r = t["xT"].rearrange("(kc p) t -> p kc t", p=128)
    outT_r = t["outT"].rearrange("(mo p) t -> p mo t", p=128)

    const = ctx.enter_context(tc.tile_pool(name="const", bufs=1))
    persist = ctx.enter_context(tc.tile_pool(name="persist", bufs=1))
    xpool = ctx.enter_context(tc.tile_pool(name="xpool", bufs=2))
    ptp = ctx.enter_context(tc.tile_pool(name="ptp", bufs=4))
    ptip = ctx.enter_context(tc.tile_pool(name="ptip", bufs=3))
    stp = ctx.enter_context(tc.tile_pool(name="stp", bufs=3))
    lrp = ctx.enter_context(tc.tile_pool(name="lrp", bufs=4))
    rbp = ctx.enter_context(tc.tile_pool(name="rbp", bufs=4))
    dramp = ctx.enter_context(tc.tile_pool(name="dramp", bufs=4, space="DRAM"))

    ps_misc = ctx.enter_context(tc.tile_pool(name="ps_misc", bufs=2, space="PSUM"))
    ps_s = ctx.enter_context(tc.tile_pool(name="ps_s", bufs=2, space="PSUM"))
    ps_o = ctx.enter_context(tc.tile_pool(name="ps_o", bufs=2, space="PSUM"))

    # per-kc weight tiles/DMAs: the first QKV matmul only waits for one
    # 128-slice of W_qkv instead of the whole transfer. Bulky non-critical
    # loads (wproj) and the outT stores ride the scalar engine's DGE queue
    # so they never delay the sync-queue x loads.
    wqkv_r = t["wqkv"].rearrange("(kc p) m -> p kc m", p=128)
    wq_sb = const.tile([128, KC, NQKV], BF16)
    for kc in range(KC):
        nc.sync.dma_start(wq_sb[:, kc, :], wqkv_r[:, kc, :])
    bias_sb = const.tile([128, 5], F32)
    nc.sync.dma_start(bias_sb[:], t["bqkv"].rearrange("m p -> p m"))
    mask_sb = const.tile([128, 1280], BF16)
    nc.scalar.dma_start(mask_sb[:], t["masks"])
    wp_sb = const.tile([128, NSLOT, C], BF16)
    nc.scalar.dma_start(wp_sb[:], t["wproj"].rearrange("h p m -> p h m"))

    KT = [persist.tile([128, T], F32R, tag=f"KT{h}", name=f"KT{h}") for h in range(HPC)]
    Vp = [
        persist.tile([128, NB, 128], BF16, tag=f"Vp{h}", name=f"Vp{h}")
        for h in range(HPC)
    ]
    # double-buffered persistent q/o tiles: the zero pad rows are written once
    # here and stay valid across chunks (only the data rows are rewritten)
    qts = [
        persist.tile([128, HPC, TQ], F32R, tag=f"qt{i}", name=f"qt{i}")
        for i in range(2)
    ]
    ots = [
        persist.tile([128, NSLOT, TQ], BF16, tag=f"ot{i}", name=f"ot{i}")
        for i in range(2)
    ]
    vsts = [
        persist.tile([128, TQ], BF16, tag=f"vst{h}", name=f"vst{h}")
        for h in range(HPC)
    ]

    for h in range(HPC):
        pad_lo = 64 - QK_BASE[h]  # 64 if data at 0, 0 if data at 64
        nc.gpsimd.memset(KT[h][pad_lo:pad_lo + 64, :].bitcast(F32), 0.0)
        if VCOL[h] == 64:
            # zero the unused low columns read by the 128-wide attnV lhsT
            nc.gpsimd.memset(Vp[h][:, :, 0:64], 0.0)
        nc.gpsimd.memset(Vp[h][:, :, ONES_COL[h]:ONES_COL[h] + 1], 1.0)
    for i in range(2):
        for h in range(HPC):
            pad_lo = 64 - QK_BASE[h]
            nc.gpsimd.memset(qts[i][pad_lo:pad_lo + 64, h, :].bitcast(F32), 0.0)
        nc.gpsimd.memset(ots[i][0:64, 1, :], 0.0)

    for j in range(NJ):
        jsl = slice(j * TQ, (j + 1) * TQ)

        # ---- QKV projection for this tq chunk ----
        xt = xpool.tile([128, KC, TQ], BF16, tag="xt")
        for kc in range(KC):
            nc.sync.dma_start(xt[:, kc, :], xT_r[:, kc, jsl])
        qt = qts[j % 2]
        vst = {}
        for m in range(5):
            ents = CHUNKS[m]
            mw = 128 if len(ents) == 2 else 64
            ps = ps_s.tile([128, 2 * TQ], F32, tag="pss", name="ps")[:, :TQ]
            for kc in range(KC):
                nc.tensor.matmul(
                    ps[:mw],
                    lhsT=wq_sb[:, kc, m * 128:m * 128 + mw],
                    rhs=xt[:, kc, :],
                    start=(kc == 0),
                    stop=(kc == KC - 1),
                )
            for (qty, h, base) in ents:
                if qty == "V":
                    vt = vsts[h]
                    vst[h] = vt
                    vb = V_BASE[h]
                    nc.scalar.activation(
                        out=vt[vb:vb + 64, :], in_=ps[vb:vb + 64, :],
                        func=IDENT, bias=bias_sb[vb:vb + 64, m:m + 1],
                    )
                else:
                    if qty == "K":
                        dst = KT[h][base:base + 64, jsl]
                    else:
                        dst = qt[base:base + 64, h, :]
                    nc.scalar.activation(
                        out=dst, in_=ps[base:base + 64, :], func=IDENT,
                        bias=bias_sb[base:base + 64, m:m + 1],
                    )

        # ---- V transposes: V.T [64, TQ] stage -> natural V in Vp, via the
        # XBAR transpose DMA (one per head per chunk; frees the PE and DVE)
        for h in range(HPC):
            vb = V_BASE[h]
            vc = VCOL[h]
            nc.sync.dma_start_transpose(
                Vp[h][:, 4 * j:4 * j + 4, vc:vc + 64],
                vst[h][vb:vb + 64, :],
            )

        # ---- streaming attention for this tq chunk ----
        ot = ots[j % 2]
        nblk = 4 * j + 4
        npair = nblk // 2
        for h in range(HPC):
            pso = ps_o.tile([128, TQ], F32, tag="pso")

            # software pipeline by one pair: emit scores(ip) one pair ahead
            # of attnV(ip) so the PE always has exp-independent work queued.
            pending = None  # (rhs_of, offs, ns, starts, (i0, i1))
            for ip in range(npair + 1):
                if ip < npair:
                    i0, i1 = 2 * ip, 2 * ip + 1
                    # column trim offsets: block i only contributes to tq
                    # columns >= 128*(i-4j) within this chunk
                    offs = [max(0, 128 * (i - 4 * j)) for i in (i0, i1)]
                    ns = [TQ - o for o in offs]
                    starts = [0, ns[0]]
                    w = ns[0] + ns[1]
                    pss = ps_s.tile([128, 2 * TQ], F32, tag="pss")
                    for n, i in enumerate((i0, i1)):
                        nc.tensor.matmul(
                            pss[:, starts[n]:starts[n] + ns[n]],
                            lhsT=KT[h][:, i * 128:(i + 1) * 128],
                            rhs=qt[:, h, offs[n]:TQ],
                            start=True,
                            stop=True,
                        )
                    rp = ip - 2 * j
                    if rp >= 0:  # diagonal pair: exact exp + packed masks
                        pt = ptp.tile([128, 2 * TQ], BF16, tag="pt")
                        nc.scalar.activation(
                            out=pt[:, :w], in_=pss[:, :w], func=EXP, scale=SCALE
                        )
                        moff = 0 if rp == 0 else 896
                        nc.vector.tensor_mul(
                            pt[:, :w], pt[:, :w], mask_sb[:, moff:moff + w]
                        )
                        rhs_of = (lambda pt: lambda s, n: pt[:, s:s + n])(pt)
                    elif SCH_ENABLE and ip % 5 < 2:  # off-diag: Schraudolph on the DVE
                        ptI = ptip.tile([128, 2 * TQ], I32, tag="ptI")
                        nc.vector.tensor_scalar(
                            out=ptI[:, :w], in0=pss[:, :w],
                            scalar1=SCH_A * SCALE, scalar2=SCH_B,
                            op0=MULT, op1=ADD,
                        )
                        rhs_of = (
                            lambda ptI: lambda s, n:
                            ptI[:, s:s + n].bitcast(BF16)[:, 1::2]
                        )(ptI)
                    else:  # off-diag: exact exp on ACT
                        pt = ptp.tile([128, 2 * TQ], BF16, tag="pt")
                        nc.scalar.activation(
                            out=pt[:, :w], in_=pss[:, :w], func=EXP, scale=SCALE
                        )
                        rhs_of = (lambda pt: lambda s, n: pt[:, s:s + n])(pt)
                    cur = (rhs_of, offs, ns, starts, (i0, i1))
                else:
                    cur = None

                if not PIPE_ENABLE:
                    pending = cur
                    cur = None
                if pending is not None:
                    (p_rhs, p_offs, p_ns, p_starts, p_blks) = pending
                    for n, i in enumerate(p_blks):
                        nc.tensor.matmul(
                            pso[0:LHS_W[h], p_offs[n]:TQ],
                            lhsT=Vp[h][:, i, LHS_LO[h]:LHS_LO[h] + LHS_W[h]],
                            rhs=p_rhs(p_starts[n], p_ns[n]),
                            start=(i == 0),
                            stop=(i == nblk - 1),
                        )
                pending = cur

            # normalize: O.T rows / denominator row (DEN_ROW[h]).
            # -1/den on the DVE via the bit-affine reciprocal seed plus one
            # Newton step: r0 = bitcast(MAGIC - bits(den)); t = den*r0;
            # lr = (t-2)*r0 = -r1. The sign is absorbed by negating W_proj
            # host-side. (DVE reciprocal costs 4us/row; ACT Ln/Exp thrash
            # the activation tables.) The partition broadcast of -1/den
            # bounces through DRAM (SBUF APs cannot have stride-0
            # partitions).
            dr = DEN_ROW[h]
            orow = O_ROW[h]
            r0i = lrp.tile([65, TQ], I32, tag="r0i")
            nc.vector.tensor_scalar(
                out=r0i[dr:dr + 1, :], in0=pso[dr:dr + 1, :].bitcast(I32),
                scalar1=-1, scalar2=RECIP_MAGIC, op0=MULT, op1=ADD,
            )
            r0 = r0i.bitcast(F32)
            td = lrp.tile([65, TQ], F32, tag="td")
            nc.vector.tensor_tensor(
                out=td[dr:dr + 1, :], in0=pso[dr:dr + 1, :],
                in1=r0[dr:dr + 1, :], op=MULT,
            )
            lr = lrp.tile([65, TQ], F32, tag="lr")
            nc.vector.scalar_tensor_tensor(
                out=lr[dr:dr + 1, :], in0=td[dr:dr + 1, :], scalar=-2.0,
                in1=r0[dr:dr + 1, :], op0=ADD, op1=MULT,
            )
            ld = dramp.tile([1, TQ], F32, tag="ld")
            nc.sync.dma_start(ld[:], lr[dr:dr + 1, :])
            rb = rbp.tile([128, TQ], F32, tag="rb")
            nc.sync.dma_start(
                rb[orow:orow + 64, :], ld[:].to_broadcast((64, TQ))
            )
            nc.vector.tensor_mul(
                ot[orow:orow + 64, OT_SLOT[h], :],
                pso[orow:orow + 64, :],
                rb[orow:orow + 64, :],
            )

        # ---- output projection, software-pipelined by one chunk ----
        # proj(j-1) is emitted here so the PE has attn(j) work to cover the
        # normalization latency of chunk j-1.
        if j > 0:
            _proj(nc, ps_misc, stp, wp_sb, outT_r, prev_ot, j - 1)
        prev_ot = ot
    _proj(nc, ps_misc, stp, wp_sb, outT_r, prev_ot, NJ - 1)


def build_nc():
    nc = bacc.Bacc("TRN2", target_bir_lowering=False, debug=False)
    t = {}
    t["xT"] = nc.dram_tensor("xT", [C, T], BF16, kind="ExternalInput").ap()
    t["wqkv"] = nc.dram_tensor("wqkv", [C, NQKV], BF16, kind="ExternalInput").ap()
    t["bqkv"] = nc.dram_tensor("bqkv", [5, 128], F32, kind="ExternalInput").ap()
    t["wproj"] = nc.dram_tensor("wproj", [NSLOT, 128, C], BF16, kind="ExternalInput").ap()
    t["masks"] = nc.dram_tensor("masks", [128, 1280], BF16, kind="ExternalInput").ap()
    t["outT"] = nc.dram_tensor("outT", [C, T], F32, kind="ExternalOutput").ap()
    with tile.TileContext(nc) as tc:
        _mhsa_body(tc, t)
    nc.compile()
    return nc


def make_in_maps(x, W_qkv, b_qkv, W_proj):
    """Shard the full inputs into one input map per core."""
    x = np.asarray(x, dtype=np.float32)
    W_qkv = np.asarray(W_qkv, dtype=np.float32)
    b_qkv = np.asarray(b_qkv, dtype=np.float32)
    W_proj = np.asarray(W_proj, dtype=np.float32)

    q_idx = np.arange(TQ)
    p_idx = np.arange(128)
    m4 = np.zeros((4, 128, TQ), dtype=np.float32)
    for r in range(4):
        m4[r] = (p_idx[:, None] <= (q_idx[None, :] - 128 * r)).astype(np.float32)
    import ml_dtypes
    masks = np.concatenate(
        [m4[0], m4[1][:, 128:], m4[2][:, 256:], m4[3][:, 384:]], axis=1
    ).astype(ml_dtypes.bfloat16)  # [128, 512+384+256+128 = 1280]

    in_maps = []
    for c in range(NCORES):
        b = c // GPB
        g = c % GPB
        heads = [HPC * g + h for h in range(HPC)]

        wg = np.zeros((C, NQKV), dtype=np.float32)
        bg = np.zeros((5, 128), dtype=np.float32)
        qty_off = {"Q": 0, "K": C, "V": 2 * C}
        for m, ents in enumerate(CHUNKS):
            for (qty, h, base) in ents:
                src = qty_off[qty] + heads[h] * D
                wg[:, m * 128 + base:m * 128 + base + D] = W_qkv[:, src:src + D]
                bg[m, base:base + D] = b_qkv[src:src + D]

        # negated: the on-core normalize computes -O/den (see kernel comment)
        # slot0 = head2 rows 0:64 + head0 rows 64:128; slot1 = head1 rows 64:128
        wp = np.zeros((NSLOT, 128, C), dtype=np.float32)
        wp[0, 0:64] = -W_proj[heads[2] * D:(heads[2] + 1) * D, :]
        wp[0, 64:128] = -W_proj[heads[0] * D:(heads[0] + 1) * D, :]
        wp[1, 64:128] = -W_proj[heads[1] * D:(heads[1] + 1) * D, :]

        in_maps.append({
            "xT": np.ascontiguousarray(x[b].T).astype(ml_dtypes.bfloat16),
            "wqkv": wg.astype(ml_dtypes.bfloat16),
            "bqkv": bg,
            "wproj": wp.astype(ml_dtypes.bfloat16),
            "masks": masks,
        })
    return in_maps


def run_cores(inputs, trace=False, **kw):
    nc = build_nc()
    in_maps = make_in_maps(
        inputs["x"], inputs["W_qkv"], inputs["b_qkv"], inputs["W_proj"]
    )
    res = run_bass_kernel_spmd(nc, in_maps, list(range(NCORES)), trace=trace, **kw)
    return res


def gather(results, b_proj):
    out = np.zeros((B, T, C), dtype=np.float32)
    for c in range(NCORES):
        out[c // GPB] += results[c]["outT"].T
    out += np.asarray(b_proj, dtype=np.float32)
    return out


def kernel(x, W_qkv, b_qkv, W_proj, b_proj):
    res = run_cores(
        {"x": x, "W_qkv": W_qkv, "b_qkv": b_qkv, "W_proj": W_proj}
    )
    return gather(res.results, b_proj)
